# revision 1
# baseline (speedup 1.0000x reference)
"""Conv4D (3^4 taps, SAME, stride 1) + bias, scaled by 1/sqrt(2).

Strategy: data-parallel over batch (B=8 -> 8 NeuronCores), weights replicated.
Per core the conv is an implicit GEMM on the TensorEngine with a z-PAIRED
stationary operand that fills the full 128x128 PE array:

  contraction K = 4 z-planes x Cin = 128 -- a contiguous window of the
    channels-first (z*ci) axis covering input planes {g..g+3};
  stationary = [128, 128]: columns 0-63 hold W taps k4=0..2 for out plane g
    (row blocks 0-2), columns 64-127 hold the same taps shifted one row block
    down for out plane g+1 (row blocks 1-3).  Both column halves share the
    same (w,x,y) alignment, so the 27 (k1,k2,k3) taps accumulate in PSUM
    exactly as in a plain conv, but each streamed column now feeds
    2 output z-planes: half the matmul instructions of the K=96/M=64 layout.

Host-side marshaling: x is zero-padded in w/x/y/z AND transposed to
channels-first [zc=576, w, x, y] so the per-group DMA is one dense transfer
with 23 KiB contiguous runs per partition.  The output is produced as
[z, co, w, x, y] on device and un-transposed on the host.  Matmuls run in
float32r (~4x fp32 throughput, rel err ~1.5e-4); operands are rounded to
f32r on the VectorEngine as the BIR verifier requires.
"""

import contextlib

import numpy as np

import concourse.bacc as bacc
import concourse.bass as bass
import concourse.mybir as mybir
import concourse.tile as tile
from concourse.bass_utils import run_bass_kernel_spmd

INV_SQRT2 = 0.7071067811865476

B = 8            # batch, one element per core
S = 16           # spatial extent in each of the 4 dims
SP = S + 2       # padded extent
CIN = 32
COUT = 64
KT = 3           # taps per dim
ZCP = SP * CIN   # padded flattened (z, ci) axis = 576
KP = 4 * CIN     # contraction size per matmul = 128 (4 z-planes)
NG = S // 2      # z-plane pairs per core = 8
NP = 8           # w-plane pairs per core

_cached = {}


MM_DT = "f32r"   # matmul operand dtype: "f32r" or "bf16"


def _build_nc(repeat=1, mm_dt=None):
    f32 = mybir.dt.float32
    mdt = {"f32r": mybir.dt.float32r, "bf16": mybir.dt.bfloat16}[mm_dt or MM_DT]
    nc = bacc.Bacc("TRN2", target_bir_lowering=False, debug=False, num_devices=B)

    x_d = nc.dram_tensor("x", (ZCP, SP, SP, SP), mdt, kind="ExternalInput")
    w_d = nc.dram_tensor("w", (KT * KT * KT, KP, 2 * COUT), mdt, kind="ExternalInput")
    b_d = nc.dram_tensor("bscaled", (2 * COUT, 1), f32, kind="ExternalInput")
    # [z-pair, 2*COUT, w, x, y] — same linear layout as [z, co, w, x, y]
    o_d = nc.dram_tensor("out", (NG, 2 * COUT, S, S, S), f32, kind="ExternalOutput")

    taps = [(k1, k2, k3) for k1 in range(KT) for k2 in range(KT) for k3 in range(KT)]

    with tile.TileContext(nc) as tc:
        with (
            tc.tile_pool(name="wpool", bufs=1) as wpool,
            tc.tile_pool(name="zpool", bufs=2) as zpool,
            tc.tile_pool(name="opool", bufs=2) as opool,
            tc.tile_pool(name="ppool", bufs=4, space=bass.MemorySpace.PSUM) as ppool,
        ):
            wt = wpool.tile([KP, KT * KT * KT, 2 * COUT], mdt)
            nc.sync.dma_start(wt[:], w_d[:].transpose([1, 0, 2]))
            bt = wpool.tile([2 * COUT, 1], f32)
            nc.sync.dma_start(bt[:], b_d[:])

            rep_ctx = (
                tc.For_i(0, repeat, 1) if repeat > 1 else contextlib.nullcontext()
            )
            with rep_ctx:
              for g in range(NG):
                zr = zpool.tile([KP, SP, SP, SP], mdt)
                nc.sync.dma_start(zr[:], x_d[2 * g * CIN : 2 * g * CIN + KP])

                ot = opool.tile([2 * COUT, S, S, S], f32)
                for p in range(NP):
                    pt = ppool.tile([2 * COUT, 2, S, S], f32)
                    for i, (k1, k2, k3) in enumerate(taps):
                        nc.tensor.matmul(
                            pt[:],
                            wt[:, (k1 * KT + k2) * KT + k3, :],
                            zr[:, 2 * p + k1 : 2 * p + k1 + 2, k2 : k2 + S, k3 : k3 + S],
                            start=(i == 0),
                            stop=(i == len(taps) - 1),
                        )
                    nc.scalar.activation(
                        ot[:, 2 * p : 2 * p + 2, :, :],
                        pt[:],
                        mybir.ActivationFunctionType.Identity,
                        bias=bt[:],
                        scale=INV_SQRT2,
                    )
                nc.gpsimd.dma_start(o_d[g], ot[:])

    nc.compile()
    return nc


def _prepare_inputs(x, W, b, mm_dt=None):
    """Marshal full (unsharded) numpy inputs into per-core in_maps."""
    mm_dt = mm_dt or MM_DT
    np_mdt = np.float32 if mm_dt == "f32r" else __import__("ml_dtypes").bfloat16
    x = np.asarray(x, dtype=np.float32)
    # pad w/x/y/z and transpose to channels-first [zc, w, x, y]
    xp = np.zeros((B, ZCP, SP, SP, SP), dtype=np.float32)
    xp[:, CIN : CIN + S * CIN, 1 : S + 1, 1 : S + 1, 1 : S + 1] = x.reshape(
        B, S, S, S, S * CIN
    ).transpose(0, 4, 1, 2, 3)

    # z-paired stationary: [27 taps, 128 rows = 4 z-planes x ci, 128 cols]
    # cols 0-63 -> out plane g (taps at row blocks 0-2)
    # cols 64-127 -> out plane g+1 (taps at row blocks 1-3)
    Wf = np.asarray(W, dtype=np.float32).reshape(KT * KT * KT, KT, CIN, COUT)
    W2 = np.zeros((KT * KT * KT, KP, 2 * COUT), dtype=np.float32)
    for k4 in range(KT):
        W2[:, k4 * CIN : (k4 + 1) * CIN, :COUT] = Wf[:, k4]
        W2[:, (k4 + 1) * CIN : (k4 + 2) * CIN, COUT:] = Wf[:, k4]
    W2 = np.ascontiguousarray(W2)

    bs = np.asarray(b, dtype=np.float32) * INV_SQRT2
    b2 = np.ascontiguousarray(np.concatenate([bs, bs]).reshape(2 * COUT, 1))

    xp = xp.astype(np_mdt)
    W2 = np.ascontiguousarray(W2.astype(np_mdt))
    return [{"x": xp[i], "w": W2, "bscaled": b2} for i in range(B)]


def kernel(x, W, b):
    if "nc" not in _cached:
        _cached["nc"] = _build_nc()
    nc = _cached["nc"]

    in_maps = _prepare_inputs(x, W, b)
    res = run_bass_kernel_spmd(nc, in_maps, core_ids=list(range(B)))
    kernel.last_exec_time_ns = res.exec_time_ns
    o_cf = np.stack(
        [res.results[i]["out"].reshape(S, COUT, S, S, S) for i in range(B)], axis=0
    )
    # [B, z, co, w, x, y] -> [B, w, x, y, z, co]
    out = np.ascontiguousarray(o_cf.transpose(0, 3, 4, 5, 1, 2))
    return out


kernel.last_exec_time_ns = None



# revision 2
# speedup vs baseline: 1.0378x; 1.0378x over previous
"""Conv4D (3^4 taps, SAME, stride 1) + bias, scaled by 1/sqrt(2).

Strategy: data-parallel over batch (B=8 -> 8 NeuronCores), weights replicated.
Per core the conv is an implicit GEMM on the TensorEngine with a z-PAIRED
stationary operand that fills the full 128x128 PE array:

  contraction K = 4 z-planes x Cin = 128 -- a contiguous window of the
    channels-first (z*ci) axis covering input planes {g..g+3};
  stationary = [128, 128]: columns 0-63 hold W taps k4=0..2 for out plane g
    (row blocks 0-2), columns 64-127 hold the same taps shifted one row block
    down for out plane g+1 (row blocks 1-3).  Both column halves share the
    same (w,x,y) alignment, so the 27 (k1,k2,k3) taps accumulate in PSUM
    exactly as in a plain conv, but each streamed column now feeds
    2 output z-planes: half the matmul instructions of the K=96/M=64 layout.

Host-side marshaling: x is zero-padded in w/x/y/z AND transposed to
channels-first [zc=576, w, x, y] so the per-group DMA is one dense transfer
with 23 KiB contiguous runs per partition.  The output is produced as
[z, co, w, x, y] on device and un-transposed on the host.  Matmuls run in
float32r (~4x fp32 throughput, rel err ~1.5e-4); operands are rounded to
f32r on the VectorEngine as the BIR verifier requires.
"""

import contextlib

import numpy as np

import concourse.bacc as bacc
import concourse.bass as bass
import concourse.mybir as mybir
import concourse.tile as tile
from concourse.bass_utils import run_bass_kernel_spmd

INV_SQRT2 = 0.7071067811865476

B = 8            # batch, one element per core
S = 16           # spatial extent in each of the 4 dims
SP = S + 2       # padded extent
CIN = 32
COUT = 64
KT = 3           # taps per dim
ZCP = SP * CIN   # padded flattened (z, ci) axis = 576
KP = 4 * CIN     # contraction size per matmul = 128 (4 z-planes)
NG = S // 2      # z-plane pairs per core = 8
NP = 8           # w-plane pairs per core

_cached = {}


MM_DT = "bf16"   # matmul operand dtype: "f32r" or "bf16"


def _build_nc(repeat=1, mm_dt=None):
    f32 = mybir.dt.float32
    mdt = {"f32r": mybir.dt.float32r, "bf16": mybir.dt.bfloat16}[mm_dt or MM_DT]
    nc = bacc.Bacc("TRN2", target_bir_lowering=False, debug=False, num_devices=B)

    x_d = nc.dram_tensor("x", (ZCP, SP, SP, SP), mdt, kind="ExternalInput")
    w_d = nc.dram_tensor("w", (KT * KT * KT, KP, 2 * COUT), mdt, kind="ExternalInput")
    b_d = nc.dram_tensor("bscaled", (2 * COUT, 1), f32, kind="ExternalInput")
    # [z-pair, 2*COUT, w, x, y] — same linear layout as [z, co, w, x, y]
    o_d = nc.dram_tensor("out", (NG, 2 * COUT, S, S, S), f32, kind="ExternalOutput")

    taps = [(k1, k2, k3) for k1 in range(KT) for k2 in range(KT) for k3 in range(KT)]

    with tile.TileContext(nc) as tc:
        with (
            tc.tile_pool(name="wpool", bufs=1) as wpool,
            tc.tile_pool(name="zpool", bufs=2) as zpool,
            tc.tile_pool(name="opool", bufs=2) as opool,
            tc.tile_pool(name="ppool", bufs=4, space=bass.MemorySpace.PSUM) as ppool,
        ):
            wt = wpool.tile([KP, KT * KT * KT, 2 * COUT], mdt)
            nc.sync.dma_start(wt[:], w_d[:].transpose([1, 0, 2]))
            bt = wpool.tile([2 * COUT, 1], f32)
            nc.sync.dma_start(bt[:], b_d[:])

            rep_ctx = (
                tc.For_i(0, repeat, 1) if repeat > 1 else contextlib.nullcontext()
            )
            with rep_ctx:
              for g in range(NG):
                zr = zpool.tile([KP, SP, SP, SP], mdt)
                nc.sync.dma_start(zr[:], x_d[2 * g * CIN : 2 * g * CIN + KP])

                ot = opool.tile([2 * COUT, S, S, S], f32)
                for p in range(NP):
                    pt = ppool.tile([2 * COUT, 2, S, S], f32)
                    for i, (k1, k2, k3) in enumerate(taps):
                        nc.tensor.matmul(
                            pt[:],
                            wt[:, (k1 * KT + k2) * KT + k3, :],
                            zr[:, 2 * p + k1 : 2 * p + k1 + 2, k2 : k2 + S, k3 : k3 + S],
                            start=(i == 0),
                            stop=(i == len(taps) - 1),
                        )
                    nc.scalar.activation(
                        ot[:, 2 * p : 2 * p + 2, :, :],
                        pt[:],
                        mybir.ActivationFunctionType.Identity,
                        bias=bt[:],
                        scale=INV_SQRT2,
                    )
                nc.gpsimd.dma_start(o_d[g], ot[:])

    nc.compile()
    return nc


def _prepare_inputs(x, W, b, mm_dt=None):
    """Marshal full (unsharded) numpy inputs into per-core in_maps."""
    mm_dt = mm_dt or MM_DT
    np_mdt = np.float32 if mm_dt == "f32r" else __import__("ml_dtypes").bfloat16
    x = np.asarray(x, dtype=np.float32)
    # pad w/x/y/z and transpose to channels-first [zc, w, x, y]
    xp = np.zeros((B, ZCP, SP, SP, SP), dtype=np.float32)
    xp[:, CIN : CIN + S * CIN, 1 : S + 1, 1 : S + 1, 1 : S + 1] = x.reshape(
        B, S, S, S, S * CIN
    ).transpose(0, 4, 1, 2, 3)

    # z-paired stationary: [27 taps, 128 rows = 4 z-planes x ci, 128 cols]
    # cols 0-63 -> out plane g (taps at row blocks 0-2)
    # cols 64-127 -> out plane g+1 (taps at row blocks 1-3)
    Wf = np.asarray(W, dtype=np.float32).reshape(KT * KT * KT, KT, CIN, COUT)
    W2 = np.zeros((KT * KT * KT, KP, 2 * COUT), dtype=np.float32)
    for k4 in range(KT):
        W2[:, k4 * CIN : (k4 + 1) * CIN, :COUT] = Wf[:, k4]
        W2[:, (k4 + 1) * CIN : (k4 + 2) * CIN, COUT:] = Wf[:, k4]
    W2 = np.ascontiguousarray(W2)

    bs = np.asarray(b, dtype=np.float32) * INV_SQRT2
    b2 = np.ascontiguousarray(np.concatenate([bs, bs]).reshape(2 * COUT, 1))

    xp = xp.astype(np_mdt)
    W2 = np.ascontiguousarray(W2.astype(np_mdt))
    return [{"x": xp[i], "w": W2, "bscaled": b2} for i in range(B)]


def kernel(x, W, b):
    if "nc" not in _cached:
        _cached["nc"] = _build_nc()
    nc = _cached["nc"]

    in_maps = _prepare_inputs(x, W, b)
    res = run_bass_kernel_spmd(nc, in_maps, core_ids=list(range(B)))
    kernel.last_exec_time_ns = res.exec_time_ns
    o_cf = np.stack(
        [res.results[i]["out"].reshape(S, COUT, S, S, S) for i in range(B)], axis=0
    )
    # [B, z, co, w, x, y] -> [B, w, x, y, z, co]
    out = np.ascontiguousarray(o_cf.transpose(0, 3, 4, 5, 1, 2))
    return out


kernel.last_exec_time_ns = None



# revision 5
# speedup vs baseline: 1.1682x; 1.1257x over previous
"""Conv4D (3^4 taps, SAME, stride 1) + bias, scaled by 1/sqrt(2).

Strategy: data-parallel over batch (B=8 -> 8 NeuronCores), weights replicated.
Per core the conv is an implicit GEMM on the TensorEngine with a z-PAIRED
stationary operand that fills the full 128x128 PE array:

  contraction K = 4 z-planes x Cin = 128 -- a contiguous window of the
    channels-first (z*ci) axis covering input planes {g..g+3};
  stationary = [128, 128]: columns 0-63 hold W taps k4=0..2 for out plane g
    (row blocks 0-2), columns 64-127 hold the same taps shifted one row block
    down for out plane g+1 (row blocks 1-3).  Both column halves share the
    same (w,x,y) alignment, so the 27 (k1,k2,k3) taps accumulate in PSUM
    exactly as in a plain conv, but each streamed column now feeds
    2 output z-planes: half the matmul instructions of the K=96/M=64 layout.

Host-side marshaling: x is zero-padded in w/x/y/z AND transposed to
channels-first [zc=576, w, x, y] so the per-group DMA is one dense transfer
with 23 KiB contiguous runs per partition.  The output is produced as
[z, co, w, x, y] on device and un-transposed on the host.  Matmuls run in
float32r (~4x fp32 throughput, rel err ~1.5e-4); operands are rounded to
f32r on the VectorEngine as the BIR verifier requires.
"""

import contextlib

import numpy as np

import concourse.bacc as bacc
import concourse.bass as bass
import concourse.mybir as mybir
import concourse.tile as tile
from concourse.bass_utils import run_bass_kernel_spmd

INV_SQRT2 = 0.7071067811865476

B = 8            # batch, one element per core
S = 16           # spatial extent in each of the 4 dims
SP = S + 2       # padded extent
CIN = 32
COUT = 64
KT = 3           # taps per dim
ZCP = SP * CIN   # padded flattened (z, ci) axis = 576
KP = 4 * CIN     # contraction size per matmul = 128 (4 z-planes)
NG = S // 2      # z-plane pairs per core = 8
NP = 8           # w-plane pairs per core

_cached = {}


MM_DT = "bf16"   # matmul operand dtype: "f32r" or "bf16"


def _build_nc(repeat=1, mm_dt=None):
    f32 = mybir.dt.float32
    mdt = {"f32r": mybir.dt.float32r, "bf16": mybir.dt.bfloat16}[mm_dt or MM_DT]
    nc = bacc.Bacc("TRN2", target_bir_lowering=False, debug=False, num_devices=B)

    x_d = nc.dram_tensor("x", (ZCP, SP, SP, SP), mdt, kind="ExternalInput")
    w_d = nc.dram_tensor("w", (KT * KT * KT, KP, 2 * COUT), mdt, kind="ExternalInput")
    b_d = nc.dram_tensor("bscaled", (2 * COUT, 1), f32, kind="ExternalInput")
    # [z-pair, 2*COUT, w, x, y] — same linear layout as [z, co, w, x, y]
    # bf16 output halves HBM write traffic; host upcasts to f32 (adds ~0.4%
    # max rounding error against the 2e-2 gate).
    o_d = nc.dram_tensor("out", (NG, 2 * COUT, S, S, S), mdt, kind="ExternalOutput")

    taps = [(k1, k2, k3) for k1 in range(KT) for k2 in range(KT) for k3 in range(KT)]

    with tile.TileContext(nc) as tc:
        with (
            tc.tile_pool(name="wpool", bufs=1) as wpool,
            tc.tile_pool(name="zpool", bufs=2) as zpool,
            tc.tile_pool(name="opool", bufs=2) as opool,
            tc.tile_pool(name="ppool", bufs=4, space=bass.MemorySpace.PSUM) as ppool,
        ):
            wt = wpool.tile([KP, KT * KT * KT, 2 * COUT], mdt)
            nc.sync.dma_start(wt[:], w_d[:].transpose([1, 0, 2]))
            bt = wpool.tile([2 * COUT, 1], f32)
            nc.sync.dma_start(bt[:], b_d[:])

            rep_ctx = (
                tc.For_i(0, repeat, 1) if repeat > 1 else contextlib.nullcontext()
            )
            with rep_ctx:
              for g in range(NG):
                zr = zpool.tile([KP, SP, SP, SP], mdt)
                nc.sync.dma_start(zr[:], x_d[2 * g * CIN : 2 * g * CIN + KP])

                ot = opool.tile([2 * COUT, S, S, S], mdt)
                for p in range(NP):
                    pt = ppool.tile([2 * COUT, 2, S, S], f32)
                    for i, (k1, k2, k3) in enumerate(taps):
                        nc.tensor.matmul(
                            pt[:],
                            wt[:, (k1 * KT + k2) * KT + k3, :],
                            zr[:, 2 * p + k1 : 2 * p + k1 + 2, k2 : k2 + S, k3 : k3 + S],
                            start=(i == 0),
                            stop=(i == len(taps) - 1),
                        )
                    nc.scalar.activation(
                        ot[:, 2 * p : 2 * p + 2, :, :],
                        pt[:],
                        mybir.ActivationFunctionType.Identity,
                        bias=bt[:],
                        scale=INV_SQRT2,
                    )
                nc.gpsimd.dma_start(o_d[g], ot[:])

    nc.compile()
    return nc


def _prepare_inputs(x, W, b, mm_dt=None):
    """Marshal full (unsharded) numpy inputs into per-core in_maps."""
    mm_dt = mm_dt or MM_DT
    np_mdt = np.float32 if mm_dt == "f32r" else __import__("ml_dtypes").bfloat16
    x = np.asarray(x, dtype=np.float32)
    # pad w/x/y/z and transpose to channels-first [zc, w, x, y]
    xp = np.zeros((B, ZCP, SP, SP, SP), dtype=np.float32)
    xp[:, CIN : CIN + S * CIN, 1 : S + 1, 1 : S + 1, 1 : S + 1] = x.reshape(
        B, S, S, S, S * CIN
    ).transpose(0, 4, 1, 2, 3)

    # z-paired stationary: [27 taps, 128 rows = 4 z-planes x ci, 128 cols]
    # cols 0-63 -> out plane g (taps at row blocks 0-2)
    # cols 64-127 -> out plane g+1 (taps at row blocks 1-3)
    Wf = np.asarray(W, dtype=np.float32).reshape(KT * KT * KT, KT, CIN, COUT)
    W2 = np.zeros((KT * KT * KT, KP, 2 * COUT), dtype=np.float32)
    for k4 in range(KT):
        W2[:, k4 * CIN : (k4 + 1) * CIN, :COUT] = Wf[:, k4]
        W2[:, (k4 + 1) * CIN : (k4 + 2) * CIN, COUT:] = Wf[:, k4]
    W2 = np.ascontiguousarray(W2)

    bs = np.asarray(b, dtype=np.float32) * INV_SQRT2
    b2 = np.ascontiguousarray(np.concatenate([bs, bs]).reshape(2 * COUT, 1))

    xp = xp.astype(np_mdt)
    W2 = np.ascontiguousarray(W2.astype(np_mdt))
    return [{"x": xp[i], "w": W2, "bscaled": b2} for i in range(B)]


def kernel(x, W, b):
    if "nc" not in _cached:
        _cached["nc"] = _build_nc()
    nc = _cached["nc"]

    in_maps = _prepare_inputs(x, W, b)
    res = run_bass_kernel_spmd(nc, in_maps, core_ids=list(range(B)))
    kernel.last_exec_time_ns = res.exec_time_ns
    o_cf = np.stack(
        [
            np.asarray(res.results[i]["out"]).astype(np.float32).reshape(S, COUT, S, S, S)
            for i in range(B)
        ],
        axis=0,
    )
    # [B, z, co, w, x, y] -> [B, w, x, y, z, co]
    out = np.ascontiguousarray(o_cf.transpose(0, 3, 4, 5, 1, 2))
    return out


kernel.last_exec_time_ns = None

